# revision 10
# baseline (speedup 1.0000x reference)
"""HGT-style GNN message passing on 8 TRN2 NeuronCores.

Strategy (per sharding hint): partition nodes + incoming edges by dst across
8 cores. Each core:
  A) projects its 12500 nodes' features (q, and fused k/v projections) on
     TensorE (node-major, via host-transposed h), writing an interleaved
     [k|v] bf16 block to DRAM in 25-tile batches,
  B) AllGathers the full 100k-node k|v table in 4 chunks (overlapped with A),
  C) per 128-node tile with degree-padded neighbor lists: ONE fused batched
     indirect DMA gather of all L neighbor k|v rows, DVE dot (bf16 tree
     reduce) + softmax (padding slots point at a zero table row -> es=1,
     v=0; corrected by subtracting the pad count from the softmax
     denominator), weighted aggregation via PE identity-matmul PSUM
     accumulation, and the output projection on TensorE. Outputs written in
     14-tile batches.
Host side does only index/layout prep (degree bucketing, edge slot tables,
inverse permutation) and the final unshard.
"""

import sys
import types
import numpy as np
from contextlib import ExitStack

import ml_dtypes

BF16 = ml_dtypes.bfloat16

N = 100000
E = 1600000
IN = 256
OUT = 128
H = 8
DK = 16
NCORES = 8
NL = N // NCORES           # 12500 nodes per core
P = 128
NTILES = (NL + P - 1) // P  # 98
SLOTS = NTILES * P          # 12544 (with ghosts)
BLK = 12800                 # rows per rank block (tiles 98,99 all-zero pad)
NCHUNK = 4                  # all-gather chunks (overlap with phase A)
CROWS = BLK // NCHUNK       # 3200 rows per chunk per rank (25 tiles)
ZBP = SLOTS                 # zero-row bucket position within a block
CS = NCORES * CROWS
TBL = BLK * NCORES
ZROW_G = (ZBP // CROWS) * CS + 0 * CROWS + (ZBP % CROWS)  # rank 0's zero row


def _prep(h, Wq, bq, Wk, bk, Wv, bv, Wmsg, bmsg, Wattn, battn, Wa, ba, src, dst):
    h = np.asarray(h, np.float32)
    src = np.asarray(src).astype(np.int64)
    dst = np.asarray(dst).astype(np.int64)
    f32 = lambda x: np.asarray(x, np.float32)
    Wq, bq, Wa, ba = f32(Wq), f32(bq), f32(Wa), f32(ba)
    Wke = f32(Wk) @ f32(Wattn)
    bke = f32(bk) @ f32(Wattn) + f32(battn)
    Wve = f32(Wv) @ f32(Wmsg)
    bve = f32(bv) @ f32(Wmsg) + f32(bmsg)

    deg = np.bincount(dst, minlength=N)
    # per-core bucketed node order (degree desc, stable)
    orders = []       # local node ids in bucket order, per core
    for c in range(NCORES):
        d = deg[c * NL:(c + 1) * NL]
        o = np.argsort(-d, kind="stable")
        orders.append(o)
    # bucket position of each node within its core block
    bpos = np.empty(N, np.int64)
    for c in range(NCORES):
        bpos[c * NL + orders[c]] = np.arange(NL)

    # shared per-tile L schedule = max degree in tile across cores (>=1)
    Ls = np.zeros(NTILES, np.int64)
    for c in range(NCORES):
        d = deg[c * NL + orders[c]]
        d = np.concatenate([d, np.zeros(SLOTS - NL, np.int64)])
        Ls = np.maximum(Ls, d.reshape(NTILES, P).max(1))
    Ls = np.maximum(Ls, 1)
    offs = np.concatenate([[0], np.cumsum(Ls * P)]).astype(np.int64)
    TOT = int(offs[-1])

    # CSR of edges by dst
    order_e = np.argsort(dst, kind="stable")
    src_s = src[order_e]
    row_off = np.concatenate([[0], np.cumsum(deg)]).astype(np.int64)

    # global kv-table row per source node (bucketed position within block)
    bp = bpos[src_s]
    tbl_row = ((bp // CROWS) * CS + (src_s // NL) * CROWS + (bp % CROWS))

    srcidxs, npads, hTs = [], [], []
    for c in range(NCORES):
        o = orders[c]
        glob = c * NL + o                                  # [NL]
        npad = np.zeros((P, NTILES), np.float32)
        dgs = deg[glob]
        blks = []
        for t in range(NTILES):
            L = int(Ls[t])
            blk = np.full((P, L), ZROW_G, np.int32)
            for r in range(P):
                p = t * P + r
                if p >= NL:
                    npad[r, t] = L - 1
                    continue
                g = glob[p]
                d = int(dgs[p])
                if d > 0:
                    blk[r, :d] = tbl_row[row_off[g]:row_off[g] + d]
                    npad[r, t] = L - d
                else:
                    npad[r, t] = L - 1
            blks.append(blk)
        si = np.ascontiguousarray(np.concatenate(blks, axis=1))  # [P, TOT//P]
        srcidxs.append(si)
        npads.append(npad)
        hT = h[glob].T                                     # [256, NL]
        hT = np.concatenate([hT, np.repeat(hT[:, :1], SLOTS - NL, 1)], 1)
        hTs.append(np.ascontiguousarray(hT.astype(BF16)))

    w = dict(
        wq=Wq.astype(BF16), wke=Wke.astype(BF16), wve=Wve.astype(BF16),
        bq=bq.reshape(1, OUT).astype(BF16), bke=bke.reshape(1, OUT).astype(BF16),
        bve=bve.reshape(1, OUT).astype(BF16),
        wa=Wa.astype(BF16), ba=ba.reshape(1, OUT).astype(BF16),
    )
    return w, hTs, srcidxs, npads, orders, Ls, offs, TOT


def _build(Ls, TOT):
    from concourse import bass, mybir, tile, bacc
    from concourse.masks import make_identity

    f32, bf16, i32 = mybir.dt.float32, mybir.dt.bfloat16, mybir.dt.int32
    nc = bacc.Bacc("TRN2", target_bir_lowering=False, debug=False,
                   enable_asserts=True, num_devices=NCORES,
                   num_swdge_queues=4)
    hT = nc.dram_tensor("hT", [IN, SLOTS], bf16, kind="ExternalInput")
    wq = nc.dram_tensor("wq", [IN, OUT], bf16, kind="ExternalInput")
    wke = nc.dram_tensor("wke", [IN, OUT], bf16, kind="ExternalInput")
    wve = nc.dram_tensor("wve", [IN, OUT], bf16, kind="ExternalInput")
    bq = nc.dram_tensor("bq", [1, OUT], bf16, kind="ExternalInput")
    bke = nc.dram_tensor("bke", [1, OUT], bf16, kind="ExternalInput")
    bve = nc.dram_tensor("bve", [1, OUT], bf16, kind="ExternalInput")
    wa = nc.dram_tensor("wa", [OUT, OUT], bf16, kind="ExternalInput")
    ba = nc.dram_tensor("ba", [1, OUT], bf16, kind="ExternalInput")
    srcidx = nc.dram_tensor("srcidx", [P, TOT // P], i32, kind="ExternalInput")
    npadf = nc.dram_tensor("npadf", [P, NTILES], f32, kind="ExternalInput")
    out = nc.dram_tensor("out", [SLOTS, OUT], f32, kind="ExternalOutput")
    kv_loc = [nc.dram_tensor(f"kv_loc{i}", [CROWS, 2 * OUT], bf16,
                             kind="Internal") for i in range(NCHUNK)]
    kv_tbl = nc.dram_tensor("kv_tbl", [TBL, 2 * OUT], bf16,
                            kind="Internal", addr_space="Shared")

    Ls = [int(x) for x in Ls]
    cum = np.concatenate([[0], np.cumsum(np.asarray(Ls))]).astype(int)
    Lmax = int(max(Ls))
    GA = 14    # hh load batch (98 = 7*14)
    GW = 25    # kv write batch = one AllGather chunk
    GO = 14    # output write batch

    with tile.TileContext(nc) as tc:
        with ExitStack() as ctx:
            const = ctx.enter_context(tc.tile_pool(name="const", bufs=1))
            sb = ctx.enter_context(tc.tile_pool(name="sb", bufs=2))
            ps = ctx.enter_context(tc.tile_pool(name="ps", bufs=2, space="PSUM"))

            identb = const.tile([P, P], bf16)
            make_identity(nc, identb[:])
            identf = const.tile([P, P], f32)
            make_identity(nc, identf[:])
            ones = const.tile([1, P], bf16)
            nc.vector.memset(ones[:], 1.0)
            # replicated weights resident in SBUF
            wq_t = const.tile([P, (IN // P) * OUT], bf16)
            wke_t = const.tile([P, (IN // P) * OUT], bf16)
            wve_t = const.tile([P, (IN // P) * OUT], bf16)
            for wt, wd in ((wq_t, wq), (wke_t, wke), (wve_t, wve)):
                for ch in range(IN // P):
                    nc.sync.dma_start(wt[:, ch * OUT:(ch + 1) * OUT],
                                      wd[ch * P:(ch + 1) * P, :])
            wa_t = const.tile([P, OUT], bf16)
            nc.sync.dma_start(wa_t[:], wa[:])
            b_t = {}
            for nm, bd in (("bq", bq), ("bke", bke), ("bve", bve), ("ba", ba)):
                b_t[nm] = const.tile([1, OUT], bf16, tag=f"b_{nm}",
                                     name=f"b_{nm}")
                nc.sync.dma_start(b_t[nm][:], bd[:])
            # q stays resident in SBUF for the whole edge phase
            q_all = const.tile([P, NTILES * OUT], bf16)
            idx_all = const.tile([P, TOT // P], i32)
            nc.sync.dma_start(idx_all[:], srcidx[:])
            npad_all = const.tile([P, NTILES], f32)
            nc.sync.dma_start(npad_all[:], npadf[:])

            # ---- phase A: projections (batched loads/writes) + chunked AG ----
            kvw = None
            for g in range(NTILES // GA):
                hh = sb.tile([P, 2 * GA * P], bf16, tag="hh")
                for ch in range(2):
                    nc.sync.dma_start(
                        hh[:, ch * GA * P:(ch + 1) * GA * P],
                        hT[ch * P:(ch + 1) * P, g * GA * P:(g + 1) * GA * P])
                for i in range(GA):
                    t = g * GA + i
                    if t % GW == 0:
                        kvw = sb.tile([P, GW * 2 * OUT], bf16, tag="kvw",
                                      bufs=1)
                    ks = t % GW
                    for wt, bn, dst_ap in (
                        (wq_t, "bq", q_all[:, t * OUT:(t + 1) * OUT]),
                        (wke_t, "bke", kvw[:, ks * 256:ks * 256 + OUT]),
                        (wve_t, "bve", kvw[:, ks * 256 + OUT:(ks + 1) * 256]),
                    ):
                        pj = ps.tile([P, OUT], f32, tag="proj", space="PSUM",
                                     bufs=2)
                        nc.tensor.matmul(out=pj[:], lhsT=hh[:, i * P:(i + 1) * P],
                                         rhs=wt[:, 0:OUT],
                                         start=True, stop=False)
                        nc.tensor.matmul(out=pj[:],
                                         lhsT=hh[:, (GA + i) * P:(GA + i + 1) * P],
                                         rhs=wt[:, OUT:2 * OUT],
                                         start=False, stop=False)
                        nc.tensor.matmul(out=pj[:], lhsT=ones[:], rhs=b_t[bn][:],
                                         start=False, stop=True)
                        nc.scalar.activation(dst_ap, pj[:],
                                             mybir.ActivationFunctionType.Copy)
                    if ks == GW - 1 or t == NTILES - 1:
                        ci = t // GW
                        if t == NTILES - 1:
                            # pad tiles 98,99 of the last chunk (incl. ZROW)
                            nc.vector.memset(
                                kvw[:, (NTILES - ci * GW) * 256:GW * 256], 0.0)
                        nc.sync.dma_start(
                            kv_loc[ci][:].rearrange("(t p) f -> p t f", p=P),
                            kvw[:].rearrange("p (t f) -> p t f", t=GW))
                        nc.gpsimd.collective_compute(
                            "AllGather", mybir.AluOpType.bypass,
                            replica_groups=[list(range(NCORES))],
                            ins=[kv_loc[ci][:]],
                            outs=[kv_tbl[ci * CS:(ci + 1) * CS, :]],
                        )

            # ---- phase C: per-tile edge compute ----
            ow = None
            for t in range(NTILES):
                L = Ls[t]
                c0 = int(cum[t])
                kvg = sb.tile([P, Lmax * 2 * OUT], bf16, tag="kvg")
                for j in range(L):
                    inst = nc.gpsimd.indirect_dma_start(
                        out=kvg[:, j * 2 * OUT:(j + 1) * 2 * OUT],
                        out_offset=None,
                        in_=kv_tbl[:],
                        in_offset=bass.IndirectOffsetOnAxis(
                            ap=idx_all[:, c0 + j:c0 + j + 1], axis=0),
                    )
                    qi = (c0 + j) % 4
                    if qi:
                        mi = getattr(inst, "ins", inst)
                        mi.queue = f"qPoolDynamic{qi}"
                kvv = kvg[:, :L * 2 * OUT].rearrange("p (j f) -> p j f", j=L)
                q_t = q_all[:, t * OUT:(t + 1) * OUT]
                # SDDMM: prod = k[src] * q[dst] (2x bf16), then tree-reduce
                prod = sb.tile([P, Lmax * OUT], bf16, tag="prod")
                p4 = prod[:, :L * OUT].rearrange("p (j h d) -> p j h d",
                                                 j=L, h=H)
                nc.vector.tensor_tensor(
                    out=prod[:, :L * OUT].rearrange("p (j f) -> p j f", j=L),
                    in0=kvv[:, :, 0:OUT],
                    in1=q_t.rearrange("p (one f) -> p one f", one=1
                                      ).to_broadcast([P, L, OUT]),
                    op=mybir.AluOpType.mult)
                es_exp = sb.tile([P, Lmax * OUT], bf16, tag="esx")
                st1 = es_exp[:, :L * 64].rearrange("p (j h d) -> p j h d",
                                                   j=L, h=H)
                nc.vector.tensor_tensor(out=st1, in0=p4[:, :, :, 0:8],
                                        in1=p4[:, :, :, 8:16],
                                        op=mybir.AluOpType.add)
                st2 = sb.tile([P, Lmax * 32], f32, tag="st2")
                v2 = st2[:, :L * 32].rearrange("p (j h d) -> p j h d",
                                               j=L, h=H)
                nc.vector.tensor_tensor(out=v2, in0=st1[:, :, :, 0:4],
                                        in1=st1[:, :, :, 4:8],
                                        op=mybir.AluOpType.add)
                st3 = sb.tile([P, Lmax * 16], f32, tag="st3")
                v3 = st3[:, :L * 16].rearrange("p (j h d) -> p j h d",
                                               j=L, h=H)
                nc.vector.tensor_tensor(out=v3, in0=v2[:, :, :, 0:2],
                                        in1=v2[:, :, :, 2:4],
                                        op=mybir.AluOpType.add)
                sS = sb.tile([P, Lmax * H], f32, tag="s")
                nc.vector.tensor_tensor(
                    out=sS[:, :L * H].rearrange("p (j h d) -> p j h d",
                                                j=L, h=H),
                    in0=v3[:, :, :, 0:1], in1=v3[:, :, :, 1:2],
                    op=mybir.AluOpType.add)
                # es -> interleaved wv|es buffer (136 cols per j)
                wvi = sb.tile([P, Lmax * 136], bf16, tag="wvi")
                wvj = wvi[:, :L * 136].rearrange("p (j f) -> p j f", j=L)
                nc.scalar.activation(
                    wvj[:, :, 128:136],
                    sS[:, :L * H].rearrange("p (j h) -> p j h", j=L),
                    mybir.ActivationFunctionType.Exp,
                    scale=1.0 / np.sqrt(DK))
                # expand es over DK on the scalar engine (frees DVE)
                nc.scalar.activation(
                    es_exp[:, :L * OUT].rearrange("p (j h d) -> p j h d",
                                                  j=L, h=H),
                    wvj[:, :, 128:136].rearrange("p j (h one) -> p j h one",
                                                 one=1
                                                 ).to_broadcast([P, L, H, DK]),
                    mybir.ActivationFunctionType.Copy)
                # wv = v * es (2x bf16, packed)
                nc.vector.tensor_tensor(
                    out=wvj[:, :, 0:OUT],
                    in0=kvv[:, :, OUT:2 * OUT],
                    in1=es_exp[:, :L * OUT].rearrange("p (j f) -> p j f", j=L),
                    op=mybir.AluOpType.mult)
                # PE identity-accumulate: agg (cols 0:128) and z (cols 128:136)
                az = ps.tile([P, 136], f32, tag="aggz", space="PSUM")
                for j in range(L):
                    nc.tensor.matmul(out=az[:], lhsT=identb[:],
                                     rhs=wvi[:, j * 136:(j + 1) * 136],
                                     start=(j == 0), stop=(j == L - 1))
                z2 = sb.tile([P, H], f32, tag="z2")
                nc.vector.tensor_scalar_sub(z2[:], az[:, 128:136],
                                            npad_all[:, t:t + 1])
                zr = sb.tile([P, H], f32, tag="zr")
                nc.vector.reciprocal(zr[:], z2[:])
                aggn = sb.tile([P, OUT], f32, tag="aggn")
                nc.vector.tensor_tensor(
                    out=aggn[:].rearrange("p (h d) -> p h d", h=H),
                    in0=az[:, 0:OUT].rearrange("p (h d) -> p h d", h=H),
                    in1=zr[:].rearrange("p (h one) -> p h one", one=1
                                        ).to_broadcast([P, H, DK]),
                    op=mybir.AluOpType.mult)
                tp = ps.tile([P, P], f32, tag="tp", space="PSUM")
                nc.tensor.transpose(out=tp[:], in_=aggn[:], identity=identf[:])
                aggT = sb.tile([P, P], bf16, tag="aggT")
                nc.scalar.activation(aggT[:], tp[:],
                                     mybir.ActivationFunctionType.Copy)
                op_ = ps.tile([P, OUT], f32, tag="op", space="PSUM")
                nc.tensor.matmul(out=op_[:], lhsT=aggT[:], rhs=wa_t[:],
                                 start=True, stop=False)
                nc.tensor.matmul(out=op_[:], lhsT=ones[:], rhs=b_t["ba"][:],
                                 start=False, stop=True)
                if t % GO == 0:
                    ow = sb.tile([P, GO * OUT], f32, tag="ow")
                nc.scalar.activation(ow[:, (t % GO) * OUT:(t % GO + 1) * OUT],
                                     op_[:],
                                     mybir.ActivationFunctionType.Copy)
                if t % GO == GO - 1 or t == NTILES - 1:
                    t0 = (t // GO) * GO
                    nt = t - t0 + 1
                    nc.sync.dma_start(
                        out[t0 * P:(t + 1) * P, :].rearrange(
                            "(t p) f -> p t f", p=P),
                        ow[:, :nt * OUT].rearrange("p (t f) -> p t f", t=nt))

    nc.compile()
    return nc


def kernel(h, Wq, bq, Wk, bk, Wv, bv, Wmsg, bmsg, Wattn, battn, Wa, ba,
           src, dst, _profile=[None]):
    from concourse.bass_utils import run_bass_kernel_spmd

    w, hTs, srcidxs, npads, orders, Ls, offs, TOT = _prep(
        h, Wq, bq, Wk, bk, Wv, bv, Wmsg, bmsg, Wattn, battn, Wa, ba, src, dst)
    nc = _build(Ls, TOT)
    in_maps = []
    for c in range(NCORES):
        m = dict(w)
        m["hT"] = hTs[c]
        m["srcidx"] = srcidxs[c]
        m["npadf"] = npads[c]
        in_maps.append(m)
    trace = _profile[0] is not None
    res = run_bass_kernel_spmd(nc, in_maps, core_ids=list(range(NCORES)),
                               trace=trace)
    if trace:
        _profile[0] = res.exec_time_ns
    full = np.empty((N, OUT), np.float32)
    for c in range(NCORES):
        oc = np.asarray(res.results[c]["out"], np.float32)
        full[c * NL + orders[c]] = oc[:NL]
    return full
